# revision 8
# baseline (speedup 1.0000x reference)
"""DifferentiableMatcher Trainium2 kernel.

cost[k, n] = 1 - <pred_k, gt_n> over HW=512*512, then 5 Sinkhorn iterations
(row/col logsumexp normalizations) and exp.

Strategy (8 NeuronCores):
  - Shard the HW contraction: core c owns HW slice [c*32768, (c+1)*32768).
  - The inputs are cast to fp16 on the host (products are exact in the fp32
    PSUM accumulate; measured end-to-end rel err ~1e-3, same order as fp32
    accumulation-order noise) which halves HBM traffic and runs the PE at
    1 cycle/row instead of fp32's dual-pass mode.
  - Host packs each shard so SBUF partition p holds runs of FB=4 HW elements
    per (q, k): packed[c, b, p, q, k, f] = x[k, h],
    h = c*32768 + b*8192 + q*512 + p*4 + f.  DMA per partition is fully
    contiguous, and the matmul reads [128, K] slices with an 8-byte stride
    (measured full-speed on PE).
  - Per core: 256 accumulating fp16 matmuls -> partial dot [100, 50] in PSUM.
  - AllReduce (20KB) across the 8 cores, then Sinkhorn runs replicated in
    fp32 log space exactly like the reference (max-subtracted logsumexp).
"""

import numpy as np

K = 100
N = 50
HW = 512 * 512
CORES = 8
SHARD = HW // CORES  # 32768
P = 128
FB = 4
Q = SHARD // (P * FB)  # 64 q-steps per core
NBLK = 8
QB = Q // NBLK  # q-steps per DMA block
TEMP = 0.1
ITERS = 5

_CACHE = {}

TRACE = False
TRACE_KW = {}
LAST_RESULT = None


def _patch_act_tables():
    """Make the combined Exp+Ln table set the only candidate for Exp/Ln so
    the compiler emits one table load instead of thrashing per activation.
    Set positions (= act_func_set_ids) are preserved."""
    import concourse.hw_specs as hw_specs
    from concourse import bacc as bacc_mod
    from concourse import mybir

    if getattr(bacc_mod, "_act_tables_patched", False):
        return
    orig = hw_specs.get_activation_tables

    def patched(arch):
        t = orig(arch)
        exp = mybir.ActivationFunctionType.Exp
        ln = mybir.ActivationFunctionType.Ln
        out = {}
        for name, funcs in t.items():
            if (exp in funcs) != (ln in funcs):
                funcs = funcs - {exp, ln}
            out[name] = funcs
        return out

    bacc_mod.get_activation_tables = patched
    bacc_mod._act_tables_patched = True


def _build():
    from concourse import bacc, tile, mybir
    from concourse.masks import make_identity

    _patch_act_tables()

    f16 = mybir.dt.float16
    f32 = mybir.dt.float32
    nc = bacc.Bacc("TRN2", target_bir_lowering=False, debug=False, enable_asserts=False, num_devices=CORES)
    p_in = nc.dram_tensor(
        "p_in", [NBLK, P, QB * K * FB], f16, kind="ExternalInput"
    ).ap()
    g_in = nc.dram_tensor(
        "g_in", [NBLK, P, QB * N * FB], f16, kind="ExternalInput"
    ).ap()
    out = nc.dram_tensor("out", [K, N], f32, kind="ExternalOutput").ap()

    with tile.TileContext(nc) as tc:
        with (
            tc.tile_pool(name="pp", bufs=NBLK) as pp,
            tc.tile_pool(name="gp", bufs=NBLK) as gp,
            tc.tile_pool(name="sk", bufs=1) as sk,
            tc.tile_pool(name="cps", bufs=1, space="PSUM") as cps,
            tc.tile_pool(name="tps", bufs=2, space="PSUM") as tps,
            tc.tile_pool(name="dram", bufs=1, space="DRAM") as dram,
        ):
            ident = sk.tile([P, P], f32)
            make_identity(nc, ident)

            C = cps.tile([K, N], f32)
            for b in range(NBLK):
                PT = pp.tile([P, QB * K * FB], f16)
                GT = gp.tile([P, QB * N * FB], f16)
                nc.scalar.dma_start(out=GT, in_=g_in[b])
                nc.sync.dma_start(out=PT, in_=p_in[b])
                PT4 = PT.rearrange("p (q k f) -> p q k f", k=K, f=FB)
                GT4 = GT.rearrange("p (q n f) -> p q n f", n=N, f=FB)
                for q in range(QB):
                    for f in range(FB):
                        nc.tensor.matmul(
                            C,
                            PT4[:, q, :, f],
                            GT4[:, q, :, f],
                            start=(b == 0 and q == 0 and f == 0),
                            stop=(b == NBLK - 1 and q == QB - 1 and f == FB - 1),
                        )

            # partial dot -> DRAM -> AllReduce -> SBUF (gpsimd ring: it is
            # idle, while sync/scalar rings are still draining input blocks)
            c_sb = sk.tile([K, N], f32)
            nc.vector.tensor_copy(out=c_sb, in_=C)
            din = dram.tile([K, N], f32)
            dout = dram.tile([K, N], f32, addr_space="Shared")
            nc.gpsimd.dma_start(out=din, in_=c_sb)
            nc.gpsimd.collective_compute(
                "AllReduce",
                mybir.AluOpType.add,
                replica_groups=[list(range(CORES))],
                ins=[din.opt()],
                outs=[dout.opt()],
            )
            csum = sk.tile([K, N], f32)
            nc.gpsimd.dma_start(out=csum, in_=dout)

            # log_alpha = -cost/TEMP = (dot - 1) / TEMP
            L = sk.tile([K, N], f32)
            nc.vector.tensor_scalar(
                out=L,
                in0=csum,
                scalar1=1.0,
                scalar2=1.0 / TEMP,
                op0=mybir.AluOpType.subtract,
                op1=mybir.AluOpType.mult,
            )
            Exp = mybir.ActivationFunctionType.Exp
            Ln = mybir.ActivationFunctionType.Ln
            cur = L  # [K, N]; SBUF first, PSUM on later iterations
            for it in range(ITERS):
                # rows (axis=2 of [1,K,N]): lse over free dim of [K, N]
                nM = sk.tile([K, 1], f32)
                nc.vector.reduce_max(
                    out=nM, in_=cur, axis=mybir.AxisListType.X, negate=True
                )
                E = sk.tile([K, N], f32)
                S = sk.tile([K, 1], f32)
                nc.scalar.activation(out=E, in_=cur, func=Exp, bias=nM, accum_out=S)
                lS = sk.tile([K, 1], f32)
                nc.scalar.activation(out=lS, in_=S, func=Ln)
                L2 = sk.tile([K, N], f32)
                nc.vector.tensor_scalar(
                    out=L2,
                    in0=cur,
                    scalar1=nM,
                    scalar2=lS,
                    op0=mybir.AluOpType.add,
                    op1=mybir.AluOpType.subtract,
                )
                # cols (axis=1): transpose, lse over free, transpose back
                TpP = tps.tile([N, K], f32)
                nc.tensor.transpose(TpP, L2, ident[:K, :K])
                nM2 = sk.tile([N, 1], f32)
                nc.vector.reduce_max(
                    out=nM2, in_=TpP, axis=mybir.AxisListType.X, negate=True
                )
                E2 = sk.tile([N, K], f32)
                S2 = sk.tile([N, 1], f32)
                nc.scalar.activation(out=E2, in_=TpP, func=Exp, bias=nM2, accum_out=S2)
                lS2 = sk.tile([N, 1], f32)
                nc.scalar.activation(out=lS2, in_=S2, func=Ln)
                Lt2 = sk.tile([N, K], f32)
                nc.vector.tensor_scalar(
                    out=Lt2,
                    in0=TpP,
                    scalar1=nM2,
                    scalar2=lS2,
                    op0=mybir.AluOpType.add,
                    op1=mybir.AluOpType.subtract,
                )
                Tp2P = tps.tile([K, N], f32)
                nc.tensor.transpose(Tp2P, Lt2, ident[:N, :N])
                cur = Tp2P

            res = sk.tile([K, N], f32)
            nc.scalar.activation(out=res, in_=cur, func=Exp)
            nc.sync.dma_start(out=out, in_=res)

    nc.compile()
    return nc


def _get_nc():
    if "nc" not in _CACHE:
        _CACHE["nc"] = _build()
    return _CACHE["nc"]


def _pack(arr, rows):
    # arr [rows, HW] fp32 -> [CORES, NBLK, P, QB*rows*FB] fp16, with
    # packed[c, b, p, (q, k, f)] = arr[k, c*SHARD + b*QB*512 + q*512 + p*FB + f]
    h = arr.astype(np.float16)
    v = h.reshape(rows, CORES, NBLK, QB, P, FB).transpose(1, 2, 4, 3, 0, 5)
    return np.ascontiguousarray(v).reshape(CORES, NBLK, P, QB * rows * FB)


def kernel(pred_masks, gt_masks):
    global LAST_RESULT
    from concourse import bass_utils

    pred = np.ascontiguousarray(np.asarray(pred_masks, dtype=np.float32)).reshape(
        K, HW
    )
    gt = np.ascontiguousarray(np.asarray(gt_masks, dtype=np.float32)).reshape(N, HW)
    pk = _pack(pred, K)
    gk = _pack(gt, N)
    in_maps = [{"p_in": pk[c], "g_in": gk[c]} for c in range(CORES)]
    nc = _get_nc()
    res = bass_utils.run_bass_kernel_spmd(
        nc, in_maps, core_ids=list(range(CORES)), trace=TRACE, **TRACE_KW
    )
    LAST_RESULT = res
    return np.asarray(res.results[0]["out"], dtype=np.float32).reshape(1, K, N)


# revision 11
# speedup vs baseline: 1.4955x; 1.4955x over previous
"""DifferentiableMatcher Trainium2 kernel.

cost[k, n] = 1 - <pred_k, gt_n> over HW=512*512, then 5 Sinkhorn iterations
(row/col logsumexp normalizations) and exp.

Strategy (8 NeuronCores):
  - Shard the HW contraction: core c owns HW slice [c*32768, (c+1)*32768).
  - The inputs are cast to fp16 on the host (products are exact in the fp32
    PSUM accumulate; measured end-to-end rel err ~1e-3, same order as fp32
    accumulation-order noise) which halves HBM traffic and runs the PE at
    1 cycle/row instead of fp32's dual-pass mode.
  - Host packs each shard so SBUF partition p holds runs of FB=4 HW elements
    per (q, k): packed[c, b, p, q, k, f] = x[k, h],
    h = c*32768 + b*8192 + q*512 + p*4 + f.  DMA per partition is fully
    contiguous, and the matmul reads [128, K] slices with an 8-byte stride
    (measured full-speed on PE).
  - Per core: 256 accumulating fp16 matmuls -> partial dot [100, 50] in PSUM.
  - AllReduce (20KB) across the 8 cores, then Sinkhorn runs replicated in
    fp32 log space exactly like the reference (max-subtracted logsumexp).
"""

import numpy as np

K = 100
N = 50
HW = 512 * 512
CORES = 8
SHARD = HW // CORES  # 32768
P = 128
FB = 4
Q = SHARD // (P * FB)  # 64 q-steps per core
NBLK = 8
QB = Q // NBLK  # q-steps per DMA block
TEMP = 0.1
ITERS = 5

_CACHE = {}

TRACE = False
TRACE_KW = {}
LAST_RESULT = None


def _patch_act_tables():
    """Make the combined Exp+Ln table set the only candidate for Exp/Ln so
    the compiler emits one table load instead of thrashing per activation.
    Set positions (= act_func_set_ids) are preserved."""
    import concourse.hw_specs as hw_specs
    from concourse import bacc as bacc_mod
    from concourse import mybir

    if getattr(bacc_mod, "_act_tables_patched", False):
        return
    orig = hw_specs.get_activation_tables

    def patched(arch):
        t = orig(arch)
        exp = mybir.ActivationFunctionType.Exp
        ln = mybir.ActivationFunctionType.Ln
        out = {}
        for name, funcs in t.items():
            if (exp in funcs) != (ln in funcs):
                funcs = funcs - {exp, ln}
            out[name] = funcs
        return out

    bacc_mod.get_activation_tables = patched
    bacc_mod._act_tables_patched = True


def _build():
    from concourse import bacc, tile, mybir
    from concourse.masks import make_identity

    _patch_act_tables()

    f16 = mybir.dt.float16
    f32 = mybir.dt.float32
    nc = bacc.Bacc("TRN2", target_bir_lowering=False, debug=False, enable_asserts=False, num_devices=CORES)
    p_in = nc.dram_tensor(
        "p_in", [NBLK, P, QB * K * FB], f16, kind="ExternalInput"
    ).ap()
    g_in = nc.dram_tensor(
        "g_in", [NBLK, P, QB * N * FB], f16, kind="ExternalInput"
    ).ap()
    out = nc.dram_tensor("out", [K, N], f32, kind="ExternalOutput").ap()

    with tile.TileContext(nc) as tc:
        with (
            tc.tile_pool(name="pp", bufs=NBLK) as pp,
            tc.tile_pool(name="gp", bufs=NBLK) as gp,
            tc.tile_pool(name="sk", bufs=1) as sk,
            tc.tile_pool(name="cps", bufs=1, space="PSUM") as cps,
            tc.tile_pool(name="tps", bufs=2, space="PSUM") as tps,
            tc.tile_pool(name="dram", bufs=1, space="DRAM") as dram,
        ):
            ident = sk.tile([P, P], f32)
            make_identity(nc, ident)

            C = cps.tile([K, N], f32)
            for b in range(NBLK):
                PT = pp.tile([P, QB * K * FB], f16)
                GT = gp.tile([P, QB * N * FB], f16)
                if b % 2 == 0:
                    nc.scalar.dma_start(out=GT, in_=g_in[b])
                    nc.sync.dma_start(out=PT, in_=p_in[b])
                else:
                    nc.sync.dma_start(out=GT, in_=g_in[b])
                    nc.scalar.dma_start(out=PT, in_=p_in[b])
                PT4 = PT.rearrange("p (q k f) -> p q k f", k=K, f=FB)
                GT4 = GT.rearrange("p (q n f) -> p q n f", n=N, f=FB)
                for q in range(QB):
                    for f in range(FB):
                        nc.tensor.matmul(
                            C,
                            PT4[:, q, :, f],
                            GT4[:, q, :, f],
                            start=(b == 0 and q == 0 and f == 0),
                            stop=(b == NBLK - 1 and q == QB - 1 and f == FB - 1),
                        )

            # partial dot -> DRAM -> AllReduce -> SBUF (gpsimd ring: it is
            # idle, while sync/scalar rings are still draining input blocks)
            c_sb = sk.tile([K, N], f32)
            nc.vector.tensor_copy(out=c_sb, in_=C)
            din = dram.tile([K, N], f32)
            dout = dram.tile([K, N], f32, addr_space="Shared")
            nc.gpsimd.dma_start(out=din, in_=c_sb)
            nc.gpsimd.collective_compute(
                "AllReduce",
                mybir.AluOpType.add,
                replica_groups=[list(range(CORES))],
                ins=[din.opt()],
                outs=[dout.opt()],
            )
            csum = sk.tile([K, N], f32)
            nc.gpsimd.dma_start(out=csum, in_=dout)

            # log_alpha = -cost/TEMP = (dot - 1) / TEMP
            L = sk.tile([K, N], f32)
            nc.vector.tensor_scalar(
                out=L,
                in0=csum,
                scalar1=1.0,
                scalar2=1.0 / TEMP,
                op0=mybir.AluOpType.subtract,
                op1=mybir.AluOpType.mult,
            )
            Exp = mybir.ActivationFunctionType.Exp
            Ln = mybir.ActivationFunctionType.Ln
            cur = L  # [K, N]; SBUF first, PSUM on later iterations
            for it in range(ITERS):
                # rows (axis=2 of [1,K,N]): lse over free dim of [K, N]
                nM = sk.tile([K, 1], f32)
                nc.vector.reduce_max(
                    out=nM, in_=cur, axis=mybir.AxisListType.X, negate=True
                )
                E = sk.tile([K, N], f32)
                S = sk.tile([K, 1], f32)
                nc.scalar.activation(out=E, in_=cur, func=Exp, bias=nM, accum_out=S)
                lS = sk.tile([K, 1], f32)
                nc.scalar.activation(out=lS, in_=S, func=Ln)
                L2 = sk.tile([K, N], f32)
                nc.vector.tensor_scalar(
                    out=L2,
                    in0=cur,
                    scalar1=nM,
                    scalar2=lS,
                    op0=mybir.AluOpType.add,
                    op1=mybir.AluOpType.subtract,
                )
                # cols (axis=1): transpose, lse over free, transpose back
                TpP = tps.tile([N, K], f32)
                nc.tensor.transpose(TpP, L2, ident[:K, :K])
                nM2 = sk.tile([N, 1], f32)
                nc.vector.reduce_max(
                    out=nM2, in_=TpP, axis=mybir.AxisListType.X, negate=True
                )
                E2 = sk.tile([N, K], f32)
                S2 = sk.tile([N, 1], f32)
                nc.scalar.activation(out=E2, in_=TpP, func=Exp, bias=nM2, accum_out=S2)
                lS2 = sk.tile([N, 1], f32)
                nc.scalar.activation(out=lS2, in_=S2, func=Ln)
                Lt2 = sk.tile([N, K], f32)
                nc.vector.tensor_scalar(
                    out=Lt2,
                    in0=TpP,
                    scalar1=nM2,
                    scalar2=lS2,
                    op0=mybir.AluOpType.add,
                    op1=mybir.AluOpType.subtract,
                )
                Tp2P = tps.tile([K, N], f32)
                nc.tensor.transpose(Tp2P, Lt2, ident[:N, :N])
                cur = Tp2P

            res = sk.tile([K, N], f32)
            nc.scalar.activation(out=res, in_=cur, func=Exp)
            nc.sync.dma_start(out=out, in_=res)

    nc.compile()
    return nc


def _get_nc():
    if "nc" not in _CACHE:
        _CACHE["nc"] = _build()
    return _CACHE["nc"]


def _pack(arr, rows):
    # arr [rows, HW] fp32 -> [CORES, NBLK, P, QB*rows*FB] fp16, with
    # packed[c, b, p, (q, k, f)] = arr[k, c*SHARD + b*QB*512 + q*512 + p*FB + f]
    h = arr.astype(np.float16)
    v = h.reshape(rows, CORES, NBLK, QB, P, FB).transpose(1, 2, 4, 3, 0, 5)
    return np.ascontiguousarray(v).reshape(CORES, NBLK, P, QB * rows * FB)


def kernel(pred_masks, gt_masks):
    global LAST_RESULT
    from concourse import bass_utils

    pred = np.ascontiguousarray(np.asarray(pred_masks, dtype=np.float32)).reshape(
        K, HW
    )
    gt = np.ascontiguousarray(np.asarray(gt_masks, dtype=np.float32)).reshape(N, HW)
    pk = _pack(pred, K)
    gk = _pack(gt, N)
    in_maps = [{"p_in": pk[c], "g_in": gk[c]} for c in range(CORES)]
    nc = _get_nc()
    res = bass_utils.run_bass_kernel_spmd(
        nc, in_maps, core_ids=list(range(CORES)), trace=TRACE, **TRACE_KW
    )
    LAST_RESULT = res
    return np.asarray(res.results[0]["out"], dtype=np.float32).reshape(1, K, N)
